# revision 6
# baseline (speedup 1.0000x reference)
"""Raw-bass pipelined TT-linear kernel (v2).

Math: W (1024x1024) is a rank-20 TT product, so
  y = (x @ Hin) @ [Hout; bias] with Hin (1024,20), Hout (20,1024).
Data-parallel over batch: 8 cores x 2048 rows.

v2 schedule notes (derived from the v1 NTFF trace):
  - 16 shared SDMA engines cap DMA at ~26 GB/s each (~420 GB/s/core);
    the goal is fat descriptors + both DGE rings busy end-to-end.
  - outputs go to DRAM in a [chunk, 128, 4096] layout (8 KiB contiguous
    per partition -> 8 KiB descriptors); the host de-transposes.
  - weights ride one packed [128, 1184] DMA (hin ++ houtb) on the gpsimd
    ring so they never queue behind the 4 MiB input stream.
  - chunk 0 is DMA'd in two column halves so GEMM1 starts ~9us.
  - PSUM p1 is zeroed once at the head; GEMM1 then uses start=True on
    each column group's first matmul, so no per-chunk re-zeroing.
    Rows between the four 20-row group slices stay zero forever (only
    GEMM1 writes p1), and t4 rows 117-127 stay zero after one head
    memset, so no NaN garbage can reach GEMM2 (houtb is zero there,
    but NaN*0 would still poison it).
  - PSUM->SBUF evacuation is split three ways (DVE/ACT/GPS) so it never
    paces the pipeline.

Engine roles:
  sync   : input DMAs (xt0 split in halves), then out-chunk DMAs 0,2
  gpsimd : weights DMA, t4 head memsets, evac share, out-chunk DMAs 1,3,
           end-of-kernel semaphore cleanup
  scalar : ACT table warm-up + evac share
  vector : p1 head memsets, t4 group copies, evac share
  tensor : matmuls, software-pipelined G1(c+1) before G2(c)
"""

from contextlib import ExitStack

import numpy as np

import concourse.bass as bass
import concourse.mybir as mybir
from concourse.bass_utils import run_bass_kernel_spmd

N_CORES = 8
B_SHARD = 2048
D_IN = 1024
D_OUT = 1024
R = 20
KC = 8
CHUNK = 512
N_CHUNKS = B_SHARD // CHUNK
BT = CHUNK // 128
QPC = 2 * BT  # half-tiles per chunk
P2_BUFS = 6
BIAS_ROW = 116
W_COLS = KC * R + D_OUT  # 160 hin cols ++ 1024 houtb cols

_DT = {"f32": mybir.dt.float32, "bf16": mybir.dt.bfloat16}

# evacuation engine per half-tile (q % 8): v=vector, s=scalar
# (gpsimd cannot access PSUM, so only DVE/ACT can evacuate)
_EVAC = "vsvsvsvs"  # 4 DVE, 4 ACT per chunk


def _eng(q):
    return _EVAC[q % 8]


def _cnt(eng, q):
    """# of halves with index <= q evacuated by `eng`."""
    return sum(1 for i in range(q + 1) if _eng(i) == eng)


def build_nc(compute="bf16", out_bf16=True):
    cdt = _DT[compute]
    odt = mybir.dt.bfloat16 if out_bf16 else mybir.dt.float32
    f32 = mybir.dt.float32

    nc = bass.Bass("TRN2", target_bir_lowering=False, debug=False)

    xt_d = nc.declare_dram_parameter(
        "xt", [N_CHUNKS, 128, KC * CHUNK], cdt, isOutput=False
    )
    wb_d = nc.declare_dram_parameter("wb", [128, W_COLS], cdt, isOutput=False)
    out_d = nc.declare_dram_parameter(
        "out", [N_CHUNKS, 128, BT * D_OUT], odt, isOutput=True
    )

    with ExitStack() as ctx:
        wb_sb = ctx.enter_context(nc.sbuf_tensor("wb_sb", [128, W_COLS], cdt))
        xt_sb = [
            ctx.enter_context(nc.sbuf_tensor(f"xt{i}", [128, KC * CHUNK], cdt))
            for i in range(N_CHUNKS)
        ]
        t4_sb = [
            ctx.enter_context(nc.sbuf_tensor(f"t4{i}", [128, CHUNK], cdt))
            for i in range(2)
        ]
        y_sb = [
            ctx.enter_context(nc.sbuf_tensor(f"y{i}", [128, BT * D_OUT], odt))
            for i in range(N_CHUNKS)
        ]
        p1 = [
            ctx.enter_context(nc.psum_tensor(f"p1{i}", [128, 512], f32))
            for i in range(2)
        ]
        p2 = [
            ctx.enter_context(nc.psum_tensor(f"p2_{i}", [128, 512], f32))
            for i in range(P2_BUFS)
        ]
        # DMA-completion semaphores: a dma_start's then_inc(sem, 16) is 16
        # independent +1s (one per SDMA engine), so only "==16 on this sem"
        # thresholds are race-free.
        sem_w = ctx.enter_context(nc.semaphore("sem_w"))
        sem_xt0a = ctx.enter_context(nc.semaphore("sem_xt0a"))
        sem_xt0b = ctx.enter_context(nc.semaphore("sem_xt0b"))
        sem_xtc = [
            ctx.enter_context(nc.semaphore(f"sem_xtc{i}"))
            for i in range(N_CHUNKS)
        ]  # index 0 unused
        sem_outc = [
            ctx.enter_context(nc.semaphore(f"sem_outc{i}"))
            for i in range(N_CHUNKS)
        ]
        # compute semaphores (single +1s in program order -> cumulative ok)
        (sem_mm1, sem_t4, sem_mm2, sem_yv, sem_ys, sem_yg, sem_p1z,
         sem_ones) = [
            ctx.enter_context(nc.semaphore(n))
            for n in (
                "sem_mm1", "sem_t4", "sem_mm2", "sem_yv", "sem_ys",
                "sem_yg", "sem_p1z", "sem_ones",
            )
        ]
        sems = (
            [sem_w, sem_xt0a, sem_xt0b]
            + sem_xtc
            + sem_outc
            + [sem_mm1, sem_t4, sem_mm2, sem_yv, sem_ys, sem_yg, sem_p1z,
               sem_ones]
        )
        nums = sorted(s.num for s in sems)
        assert nums == list(range(nums[0], nums[0] + len(nums))), nums
        sem_range = range(nums[0], nums[-1] + 1)

        sem_of = {"v": sem_yv, "s": sem_ys, "g": sem_yg}

        def evac_wait(engine, q):
            """Wait until evacuation of half-tile q has completed."""
            engine.wait_ge(sem_of[_eng(q)], _cnt(_eng(q), q))

        def all_evac_wait(engine, q):
            """Wait until every half-tile <= q has been evacuated."""
            for e in "vs":
                n = _cnt(e, q)
                if n:
                    engine.wait_ge(sem_of[e], n)

        with nc.Block() as block:

            @block.sync
            def _(sync):
                # chunk-0 column halves lead so GEMM1 starts early
                sync.dma_start(
                    out=xt_sb[0][:, 0 : 4 * CHUNK], in_=xt_d[0][:, 0 : 4 * CHUNK]
                ).then_inc(sem_xt0a, 16)
                sync.dma_start(
                    out=xt_sb[0][:, 4 * CHUNK :], in_=xt_d[0][:, 4 * CHUNK :]
                ).then_inc(sem_xt0b, 16)
                for c in range(1, N_CHUNKS):
                    sync.dma_start(out=xt_sb[c][:], in_=xt_d[c]).then_inc(
                        sem_xtc[c], 16
                    )
                for c in (0, 2):
                    all_evac_wait(sync, QPC * c + QPC - 1)
                    sync.dma_start(out=out_d[c], in_=y_sb[c][:]).then_inc(
                        sem_outc[c], 16
                    )

            @block.tensor
            def _(tensor):
                def g1(c):
                    # four column groups concurrent (tile_position=(0,32j));
                    # kc<4 overwrites (start=True), kc>=4 accumulates
                    for kc in range(KC):
                        j = kc % 4
                        if kc == 0:
                            if c == 0:
                                tensor.wait_ge(sem_w, 16)
                                tensor.wait_ge(sem_xt0a, 16)
                            else:
                                tensor.wait_ge(sem_xtc[c], 16)
                            if c < 2:
                                tensor.wait_ge(sem_p1z, c + 1)
                            else:
                                # start=True overwrite must not race
                                # t4copy(c-2)'s read of this bank
                                tensor.wait_ge(sem_t4, c - 1)
                        if c == 0 and kc == 4:
                            tensor.wait_ge(sem_xt0b, 16)
                        mm = tensor.matmul(
                            p1[c % 2][32 * j : 32 * j + R, 0:CHUNK],
                            wb_sb[:, kc * R : (kc + 1) * R],
                            xt_sb[c][:, kc * CHUNK : (kc + 1) * CHUNK],
                            start=(kc < 4),
                            stop=(kc == KC - 1),
                            tile_position=(0, 32 * j),
                            skip_group_check=True,
                        )
                        if kc == KC - 1:
                            mm.then_inc(sem_mm1)

                def g2(c):
                    for bt in range(BT):
                        for nh in range(2):
                            q = QPC * c + 2 * bt + nh
                            if q == QPC * c:
                                tensor.wait_ge(sem_t4, c + 1)
                                if c < 2:
                                    tensor.wait_ge(sem_ones, c + 1)
                            if q >= P2_BUFS:
                                evac_wait(tensor, q - P2_BUFS)
                            tensor.matmul(
                                p2[q % P2_BUFS][:],
                                t4_sb[c % 2][:, bt * 128 : (bt + 1) * 128],
                                wb_sb[:, KC * R + nh * 512 : KC * R + (nh + 1) * 512],
                                start=True,
                                stop=True,
                            ).then_inc(sem_mm2)

                for c in range(N_CHUNKS):
                    g1(c)
                    g2(c)

            @block.vector
            def _(vector):
                # one-time p1 zeroing: group-gap rows must stay exactly 0
                # (PSUM garbage could be NaN; NaN*0 poisons GEMM2)
                vector.memset(p1[0][:], 0.0).then_inc(sem_p1z)
                vector.memset(p1[1][:], 0.0).then_inc(sem_p1z)

                def t4copy(c):
                    vector.wait_ge(sem_mm1, c + 1)
                    if c < 2:
                        vector.wait_ge(sem_ones, c + 1)  # head memsets done
                    else:
                        # t4 buffer reuse: all GEMM2 of chunk c-2 done
                        vector.wait_ge(sem_mm2, QPC * (c - 2) + QPC)
                    vector.tensor_copy(
                        t4_sb[c % 2][0:BIAS_ROW, :],
                        p1[c % 2][0:BIAS_ROW, 0:CHUNK],
                    ).then_inc(sem_t4)

                def evacs(c):
                    for bt in range(BT):
                        for nh in range(2):
                            q = QPC * c + 2 * bt + nh
                            if _eng(q) != "v":
                                continue
                            vector.wait_ge(sem_mm2, q + 1)
                            o0 = bt * D_OUT + nh * 512
                            vector.tensor_copy(
                                y_sb[c][:, o0 : o0 + 512],
                                p2[q % P2_BUFS][:],
                            ).then_inc(sem_yv)

                for c in range(N_CHUNKS):
                    t4copy(c)
                    evacs(c)

            @block.scalar
            def _(scalar):
                # dummy copy: pull the one-time ACT_TABLE_LOAD (~1.3us) into
                # the head instead of the first real evacuation
                scalar.wait_ge(sem_ones, 1)
                scalar.copy(y_sb[0][0:1, 0:32], t4_sb[0][0:1, 0:32])
                for c in range(N_CHUNKS):
                    for bt in range(BT):
                        for nh in range(2):
                            q = QPC * c + 2 * bt + nh
                            if _eng(q) != "s":
                                continue
                            scalar.wait_ge(sem_mm2, q + 1)
                            o0 = bt * D_OUT + nh * 512
                            scalar.copy(
                                y_sb[c][:, o0 : o0 + 512],
                                p2[q % P2_BUFS][:],
                            ).then_inc(sem_ys)

            @block.gpsimd
            def _(gpsimd):
                # weights first: gates GEMM1(0)
                gpsimd.dma_start(out=wb_sb[:], in_=wb_d[:]).then_inc(sem_w, 16)
                # t4 rows 96-127 <- 1.0 once (partition base must be 32-
                # aligned): row 116 is the bias/ones row; rows 96-115 are
                # re-written by every t4copy before GEMM2 reads them; rows
                # 117-127 hit zero houtb rows (1.0, not garbage, so no NaN).
                # Rows 0-95 are fully written by t4copy each chunk.
                for i in range(2):
                    gpsimd.memset(t4_sb[i][96:128, :], 1.0).then_inc(sem_ones)
                for c in (1, 3):
                    all_evac_wait(gpsimd, QPC * c + QPC - 1)
                    gpsimd.dma_start(out=out_d[c], in_=y_sb[c][:]).then_inc(
                        sem_outc[c], 16
                    )
                for c in range(N_CHUNKS):
                    gpsimd.wait_ge(sem_outc[c], 16)
                # leave semaphores clean for any re-execution
                gpsimd.dma_reset(sem_range)
                gpsimd.sem_clear(sem_range)

    return nc


def host_prep(x, cores, bias, np_dt):
    A = cores[0][0].astype(np.float64)
    for G in cores[1:4]:
        G = G.astype(np.float64)
        A = np.einsum("ir,rjs->ijs", A, G).reshape(-1, G.shape[2])
    H = cores[4].astype(np.float64)
    for G in cores[5:]:
        G = G.astype(np.float64)
        H = np.einsum("pNq,qnr->pNnr", H, G).reshape(H.shape[0], -1, G.shape[2])
    H = H.reshape(H.shape[0], -1)  # (20, 1024)

    hin = np.ascontiguousarray(
        A.reshape(KC, 128, R).transpose(1, 0, 2).reshape(128, KC * R)
    )
    # Hout replicated into the four 32-row column groups + bias in row 116;
    # rows outside the rank blocks stay exactly 0 (t4 garbage protection)
    houtb = np.zeros((128, D_OUT), dtype=np.float64)
    for j in range(4):
        houtb[32 * j : 32 * j + R, :] = H
    houtb[BIAS_ROW, :] = bias.astype(np.float64)
    wb = np.concatenate([hin, houtb], axis=1).astype(np_dt)  # [128, 1184]
    xt = np.ascontiguousarray(
        x.reshape(N_CORES, N_CHUNKS, CHUNK, KC, 128).transpose(0, 1, 4, 3, 2)
    ).astype(np_dt).reshape(N_CORES, N_CHUNKS, 128, KC * CHUNK)
    return xt, wb


def unshard_out(raw):
    """[N_CHUNKS, 128, BT*D_OUT] -> [B_SHARD, D_OUT]"""
    return (
        raw.reshape(N_CHUNKS, 128, BT, D_OUT)
        .transpose(0, 2, 1, 3)
        .reshape(B_SHARD, D_OUT)
    )


_NC_CACHE = {}


def run(x, cores, bias, compute="bf16", out_bf16=True, trace=False):
    np_dt = np.dtype(mybir.dt.np(_DT[compute]))
    xt, wb = host_prep(x, cores, bias, np_dt)
    key = (compute, out_bf16)
    if key not in _NC_CACHE:
        _NC_CACHE[key] = build_nc(compute, out_bf16)
    nc = _NC_CACHE[key]
    in_maps = [{"xt": xt[i], "wb": wb} for i in range(N_CORES)]
    res = run_bass_kernel_spmd(nc, in_maps, list(range(N_CORES)), trace=trace)
    out = np.concatenate(
        [unshard_out(res.results[i]["out"]) for i in range(N_CORES)], axis=0
    )
    return out.astype(np.float32), res


def kernel(x, core0, core1, core2, core3, core4, core5, core6, core7, bias):
    cores = (core0, core1, core2, core3, core4, core5, core6, core7)
    out, _ = run(
        np.asarray(x, dtype=np.float32),
        [np.asarray(c, dtype=np.float32) for c in cores],
        np.asarray(bias, dtype=np.float32),
    )
    return out
